# revision 1
# baseline (speedup 1.0000x reference)
"""AxialSoftAttention kernel for 8 Trainium2 NeuronCores.

Sharding: 8 cores = 4 batch elements x 2 frequency halves (data-parallel,
comms-free). Each core receives the full x[b] (rolled so its own 128
frequency rows sit first), computes:
  - fqkv conv (BN folded into weights on host) for all 256 freq rows
    (kf, v need full F), qf only for its own half,
  - frequency-axis attention for its own 128 f-rows,
  - causal time-axis attention (qt/kt conv on own half only),
  - proj conv + residual,
and returns its [64, 128, 384] output slab. Host reassembles the full
[4, 64, 256, 384] output.

BatchNorm (inference) + the 1/sqrt(DC) attention scale are folded into the
conv weights/biases on the host: PReLU is positive-homogeneous, so scaling
the q-rows of the conv weight by 0.25 pre-activation equals scaling q
post-activation.
"""

import numpy as np

C = 64
DC = 16
F = 256
F2 = 128
T = 384
B = 4
EPS = 1e-5

_jitted = None


def _build():
    import jax
    import jax.numpy as jnp
    from functools import partial

    @partial(jax.jit, donate_argnums=(0,))
    def shard(x, W1, b1, a1, W2, b2, a2, W3, b3, a3):
        # x: [64, 256, 384] fp32, own f-half rolled to rows 0:128
        f32 = jnp.float32

        def prelu(z, a):
            return jnp.where(z >= 0, z, a[:, None, None] * z)

        with jax.default_matmul_precision("highest"):
            fqkv = prelu(jnp.einsum("oc,cft->oft", W1, x) + b1[:, None, None], a1)
            qf = fqkv[:DC, :F2]          # [16,128,384] (0.25 scale folded in W1)
            kf = fqkv[DC:2 * DC]         # [16,256,384]
            v = fqkv[2 * DC:]            # [16,256,384]

            fs = jnp.einsum("cft,cyt->tfy", qf, kf)          # [384,128,256]
            fs = jax.nn.softmax(fs, axis=-1)
            fo = jnp.einsum("tfy,cyt->cft", fs, v)           # [16,128,384]

            x_own = x[:, :F2]
            tqk = prelu(jnp.einsum("oc,cft->oft", W2, x_own) + b2[:, None, None], a2)
            qt = tqk[:DC]                # [16,128,384] (0.25 folded)
            kt = tqk[DC:]

            ts = jnp.einsum("cft,cfy->fty", qt, kt)          # [128,384,384]
            mask = jnp.triu(jnp.ones((T, T), bool), k=1)
            ts = jnp.where(mask[None], -jnp.finfo(f32).max, ts)
            ts = jax.nn.softmax(ts, axis=-1)
            to = jnp.einsum("fty,cfy->cft", ts, fo)          # [16,128,384]

            out = prelu(jnp.einsum("oc,cft->oft", W3, to) + b3[:, None, None], a3)
            return out + x_own

    return shard


def kernel(x, fqkv_w, fqkv_gamma, fqkv_beta, fqkv_mean, fqkv_var, fqkv_alpha,
           tqk_w, tqk_gamma, tqk_beta, tqk_mean, tqk_var, tqk_alpha,
           proj_w, proj_gamma, proj_beta, proj_mean, proj_var, proj_alpha):
    import jax

    global _jitted
    if _jitted is None:
        _jitted = _build()
    shard = _jitted

    # ---- host-side BN folding ----
    def fold(w, gamma, beta, mean, var):
        s = gamma / np.sqrt(var + EPS)
        return (s[:, None] * w).astype(np.float32), (beta - s * mean).astype(np.float32)

    W1, b1 = fold(np.asarray(fqkv_w), np.asarray(fqkv_gamma), np.asarray(fqkv_beta),
                  np.asarray(fqkv_mean), np.asarray(fqkv_var))
    W2, b2 = fold(np.asarray(tqk_w), np.asarray(tqk_gamma), np.asarray(tqk_beta),
                  np.asarray(tqk_mean), np.asarray(tqk_var))
    W3, b3 = fold(np.asarray(proj_w), np.asarray(proj_gamma), np.asarray(proj_beta),
                  np.asarray(proj_mean), np.asarray(proj_var))
    # fold attention scale 1/sqrt(DC)=0.25 into the q rows (PReLU is
    # positive-homogeneous, so this commutes with the activation)
    W1 = W1.copy(); b1 = b1.copy()
    W1[:DC] *= 0.25; b1[:DC] *= 0.25
    W2 = W2.copy(); b2 = b2.copy()
    W2[:DC] *= 0.25; b2[:DC] *= 0.25
    a1 = np.asarray(fqkv_alpha, np.float32)
    a2 = np.asarray(tqk_alpha, np.float32)
    a3 = np.asarray(proj_alpha, np.float32)

    x = np.asarray(x, np.float32)
    devs = jax.devices()[:8]

    weights = (W1, b1, a1, W2, b2, a2, W3, b3, a3)
    futures = []
    for k in range(8):
        b, fh = divmod(k, 2)
        xk = np.roll(x[b], -fh * F2, axis=1)  # own half first
        dev = devs[k]
        args = [jax.device_put(xk, dev)] + [jax.device_put(w, dev) for w in weights]
        futures.append(shard(*args))

    out = np.empty((B, C, F, T), np.float32)
    for k, fut in enumerate(futures):
        b, fh = divmod(k, 2)
        out[b, :, fh * F2:(fh + 1) * F2, :] = np.asarray(fut)
    return out

